# revision 33
# baseline (speedup 1.0000x reference)
"""Trainium2 Bass kernel for nn_Apply_on_single_area.

Computes, per supervoxel area b:
    loss[b] = sum_{i,j} eroded(mc)[i,j] * em[i,j]
where mc = mask_combined[..., mask_index] with last row/col zeroed and
eroded = E(a1)*E(a2), E(a) = 2a - a^2, a1/a2 = products with the next
element along each spatial axis (zero-padded).

Key simplifications / design (HW-measured on TRN2):
- differentiable_or_simple(a,b) = a*b + (1-a)*a + (1-b)*a = 2a - a^2:
  the b-terms cancel, so only forward-neighbor products a1, a2 matter.
- Flattening (i,j) -> k = 32i+j turns the spatial shifts into flat
  shifts +32/+1; zeroing row/col 31 of mc makes that exact.
- Pure data parallel: B=10000 split 1250/core over 8 cores, padded to
  1280 = 128 partitions x 10 areas, partition-major so every DMA is
  contiguous per partition.
- bf16 compute (inputs converted host-side; rel err ~1e-3 vs the 2e-2
  gate), f32 accumulation for the per-area sums.
- Engine split: DVE does the same-tensor shifted products t1,t2 (2x
  mode), the fused e12 = 1-u12 tensor_scalar, p = e1*e2 and w = p*em;
  ACT does one fused Square for u12 = (1-t12)^2 and the per-area
  Copy+accum reductions. GPSIMD is deliberately unused - any gpsimd
  involvement measured ~10us slower. Emission is lag-pipelined
  (stage_a/b/c offset by supertile) so neither engine stalls on
  same-supertile cross-engine dependencies.
"""

import numpy as np

import jax
from jax.experimental.shard_map import shard_map
from jax.sharding import Mesh, NamedSharding, PartitionSpec

import concourse.bass as bass
import concourse.bacc as bacc
import concourse.mybir as mybir
import concourse.tile as tile
from concourse import bass2jax

N_CORES = 8
B_TOTAL = 10000
SHARD = B_TOTAL // N_CORES  # 1250
C_PER_P = 10  # areas per partition (after padding shard to 1280)
SHARD_PAD = 128 * C_PER_P
AREA = 1024  # 32*32
W = 32
NV = AREA - W  # 992 valid flat positions (rows 0..30)

F32 = mybir.dt.float32
BF16 = mybir.dt.bfloat16

_NC_CACHE = {}


def _supertiles(shard: int, A: int):
    """Split `shard` areas into supertiles (base, P, a) with a area-slots of
    P partitions each. Area index = base + 128*j + p for slot j, partition p."""
    out = []
    base = 0
    while shard - base >= 128 * A:
        out.append((base, 128, A))
        base += 128 * A
    while shard - base >= 128:
        out.append((base, 128, 1))
        base += 128
    if shard > base:
        out.append((base, shard - base, 1))
        base = shard
    return out


def _build(shard: int, inner_reps: int = 1, A: int = 2, variant: str = "v10-k0") -> bass.Bass:
    """Per-core SPMD graph: mc [1280,1024] bf16 (edges pre-zeroed, rows
    1250..1279 zero-padded), em [1280,992] bf16 -> out [1280] f32.

    Partition-major layout: area = p*C_PER_P + t, so every DMA is
    contiguous per partition (loads 2-4 KB lines, store one 40 B line).

    Math: loss = sum_k e(t1)*e(t2)*em with e(t) = t*(2-t) = 1-(1-t)^2,
    t1[k]=m[k]*m[k+32], t2[k]=m[k]*m[k+1] over k in [0,992).

    Two-engine split (HW-measured): DVE t1/t2 (same-tensor shifted tt),
    e=1-u (ts), p=e1*e2, w=p*em (tt); ACT squares u=(1-t)^2 and the
    final Copy+accum reduction per area. Lag-pipelined emission."""
    assert shard == SHARD_PAD, shard
    C = C_PER_P
    nc = bacc.Bacc("TRN2", target_bir_lowering=False, debug=False)

    mc_d = nc.declare_dram_parameter("mc", [shard, AREA], BF16, isOutput=False)
    em_d = nc.declare_dram_parameter("em", [shard, NV], BF16, isOutput=False)
    out_d = nc.declare_dram_parameter("out", [shard], F32, isOutput=True)

    n_super = C // A
    AL = mybir.AluOpType
    AF = mybir.ActivationFunctionType
    mc_v = mc_d.ap().rearrange("(p c) k -> p c k", c=C)
    em_v = em_d.ap().rearrange("(p c) k -> p c k", c=C)

    reuse = "reuse" in variant or "bufs6" in variant
    mid_bufs = 6 if "bufs6" in variant else 4
    lag_c = 3 if "lag3" in variant else 2
    with tile.TileContext(nc) as tc:
        with (
            tc.tile_pool(name="ld", bufs=4) as ld,
            tc.tile_pool(name="mid", bufs=mid_bufs) as mid,
            tc.tile_pool(name="res", bufs=4) as resp,
        ):
          for _rr in range(inner_reps):
            res_t = resp.tile([128, C], F32, tag="res")
            stage_state = {}

            def stage_a(s):
                c0 = s * A
                m = ld.tile([128, A, AREA], BF16, tag="m")
                e = ld.tile([128, A, NV], BF16, tag="e")
                if variant == "tinydma":
                    nc.sync.dma_start(out=m[:, :, 0:16], in_=mc_v[:, c0 : c0 + A, 0:16])
                    nc.sync.dma_start(out=e[:, :, 0:16], in_=em_v[:, c0 : c0 + A, 0:16])
                else:
                    nc.sync.dma_start(out=m[:], in_=mc_v[:, c0 : c0 + A, :])
                    nc.sync.dma_start(out=e[:], in_=em_v[:, c0 : c0 + A, :])
                if variant == "dmaonly":
                    stage_state[s] = (e, None, None)
                    return
                if "v10" in variant:
                    t12 = mid.tile([128, 2, A, NV], BF16, tag="t12")
                    nc.vector.tensor_tensor(
                        t12[:, 0], m[:, :, 0:NV], m[:, :, W:AREA], AL.mult
                    )
                    nc.vector.tensor_tensor(
                        t12[:, 1], m[:, :, 0:NV], m[:, :, 1 : 1 + NV], AL.mult
                    )
                    stage_state[s] = (e, t12, None)
                    return
                t1 = mid.tile([128, A, NV], BF16, tag="t1")
                nc.vector.tensor_tensor(t1[:], m[:, :, 0:NV], m[:, :, W:AREA], AL.mult)
                t2 = mid.tile([128, A, NV], BF16, tag="t2")
                nc.vector.tensor_tensor(t2[:], m[:, :, 0:NV], m[:, :, 1 : 1 + NV], AL.mult)
                stage_state[s] = (e, t1, t2)

            def stage_b(s):
                if variant == "dmaonly":
                    return
                e, t1, t2 = stage_state[s]
                if "v10" in variant:
                    t12 = t1
                    u12 = mid.tile([128, 2, A, NV], BF16, tag="u12")
                    nc.scalar.activation(u12[:], t12[:], AF.Square, bias=1.0, scale=-1.0)
                    e12 = mid.tile([128, 2, A, NV], BF16, tag="e12")
                    nc.vector.tensor_scalar(
                        e12[:], u12[:], -1.0, 1.0, op0=AL.mult, op1=AL.add
                    )
                    stage_state[s] = (e, e12, None)
                    return
                u1 = mid.tile([128, A, NV], BF16, tag="u1")
                nc.scalar.activation(u1[:], t1[:], AF.Square, bias=1.0, scale=-1.0)
                u2 = mid.tile([128, A, NV], BF16, tag="u2")
                nc.scalar.activation(u2[:], t2[:], AF.Square, bias=1.0, scale=-1.0)
                if "v9" in variant:
                    # e1,e2 share one tile so p = e1*e2 hits the DVE
                    # single-source fastpath
                    e12 = mid.tile([128, 2, A, NV], BF16, tag="e12")
                    nc.vector.tensor_scalar(
                        e12[:, 0], u1[:], -1.0, 1.0, op0=AL.mult, op1=AL.add
                    )
                    nc.vector.tensor_scalar(
                        e12[:, 1], u2[:], -1.0, 1.0, op0=AL.mult, op1=AL.add
                    )
                    stage_state[s] = (e, e12, None)
                    return
                e1 = mid.tile([128, A, NV], BF16, tag="t1" if reuse else "e1")
                nc.vector.tensor_scalar(e1[:], u1[:], -1.0, 1.0, op0=AL.mult, op1=AL.add)
                e2 = mid.tile([128, A, NV], BF16, tag="t2" if reuse else "e2")
                nc.vector.tensor_scalar(e2[:], u2[:], -1.0, 1.0, op0=AL.mult, op1=AL.add)
                stage_state[s] = (e, e1, e2)

            def stage_c(s):
                c0 = s * A
                e, e1, e2 = stage_state.pop(s)
                if variant == "dmaonly":
                    return
                if "v9" in variant or "v10" in variant:
                    e12 = e1
                    p_t = mid.tile([128, A, NV], BF16, tag="p")
                    nc.vector.tensor_tensor(p_t[:], e12[:, 0], e12[:, 1], AL.mult)
                    w = mid.tile([128, A, NV], BF16, tag="w")
                    nc.vector.tensor_tensor(w[:], p_t[:], e[:], AL.mult)
                    if "fold1" in variant:
                        # halve reduction length with one same-tile tt-add
                        wh = mid.tile([128, A, NV // 2], BF16, tag="wh")
                        nc.vector.tensor_tensor(
                            wh[:], w[:, :, 0 : NV // 2], w[:, :, NV // 2 : NV], AL.add
                        )
                        for j in range(A):
                            dum = mid.tile([128, NV // 2], BF16, tag="dumh")
                            nc.scalar.activation(
                                dum[:], wh[:, j], AF.Copy,
                                accum_out=res_t[:, c0 + j : c0 + j + 1],
                            )
                        return
                    for j in range(A):
                        if "k0" not in variant and s in (1, 3) and j == 0:
                            qd = mid.tile([128, NV], BF16, tag="dum")
                            nc.vector.tensor_scalar(
                                qd[:], w[:, j], 1.0, None, op0=AL.mult, op1=AL.add,
                                accum_out=res_t[:, c0 + j : c0 + j + 1],
                            )
                        else:
                            dum = mid.tile([128, NV], BF16, tag="dum")
                            nc.scalar.activation(
                                dum[:], w[:, j], AF.Copy,
                                accum_out=res_t[:, c0 + j : c0 + j + 1],
                            )
                    return
                p_t = mid.tile([128, A, NV], BF16, tag="u1" if reuse else "p")
                nc.vector.tensor_tensor(p_t[:], e1[:], e2[:], AL.mult)
                if "v8" in variant:
                    # w = p*em on DVE, then DMA-CCE tree-fold (add) 992->124
                    # so the per-area reductions touch only 124 elements
                    w = mid.tile([128, A, NV], BF16, tag="u2" if reuse else "w")
                    nc.vector.tensor_tensor(w[:], p_t[:], e[:], AL.mult)
                    nc.gpsimd.dma_start(
                        out=w[:, :, 0:496], in_=w[:, :, 496:992], accum_op=AL.add
                    )
                    nc.gpsimd.dma_start(
                        out=w[:, :, 0:248], in_=w[:, :, 248:496], accum_op=AL.add
                    )
                    nc.gpsimd.dma_start(
                        out=w[:, :, 0:124], in_=w[:, :, 124:248], accum_op=AL.add
                    )
                    for j in range(A):
                        dum = mid.tile([128, 124], BF16, tag="dum8")
                        nc.scalar.activation(
                            dum[:], w[:, j, 0:124], AF.Copy,
                            accum_out=res_t[:, c0 + j : c0 + j + 1],
                        )
                    return
                if "v7" in variant:
                    # w = p*em computed by the DMA compute engine, in-place
                    # into the em tile; reductions split DVE/ACT per slot
                    nc.gpsimd.dma_start(out=e[:], in_=p_t[:], accum_op=AL.mult)
                    for j in range(A):
                        if j % 2 == 0:
                            qd = mid.tile([128, NV], BF16, tag="dum")
                            nc.vector.tensor_scalar(
                                qd[:], e[:, j], 1.0, None, op0=AL.mult,
                                op1=AL.add,
                                accum_out=res_t[:, c0 + j : c0 + j + 1],
                            )
                        else:
                            dum = mid.tile([128, NV], BF16, tag="dum")
                            nc.scalar.activation(
                                dum[:], e[:, j], AF.Copy,
                                accum_out=res_t[:, c0 + j : c0 + j + 1],
                            )
                    return
                w = mid.tile([128, A, NV], BF16, tag="u2" if reuse else "w")
                if "gw" in variant:
                    nc.gpsimd.tensor_tensor(w[:], p_t[:], e[:], AL.mult)
                else:
                    nc.vector.tensor_tensor(w[:], p_t[:], e[:], AL.mult)
                for j in range(A):
                    dum = mid.tile([128, NV], BF16, tag="dum")
                    nc.scalar.activation(
                        dum[:], w[:, j], AF.Copy,
                        accum_out=res_t[:, c0 + j : c0 + j + 1],
                    )

            for s in range(n_super + lag_c):
                if s < n_super:
                    stage_a(s)
                if 1 <= s < n_super + 1:
                    stage_b(s - 1)
                if s >= lag_c:
                    stage_c(s - lag_c)

            if variant == "dmaonly":
                nc.gpsimd.memset(res_t[:], 0.0)
            if variant != "lastout" or _rr == inner_reps - 1:
                nc.sync.dma_start(
                    out=out_d.ap().rearrange("(p c) -> p c", c=C), in_=res_t[:]
                )

    nc.compile()
    return nc

class _Exec:
    """One-time-jitted SPMD executor for a prebuilt Bass graph.

    Vendored from bass2jax.run_bass_via_pjrt so repeated calls reuse the
    compiled executable (run_bass_via_pjrt re-jits per invocation)."""

    def __init__(self, nc: bass.Bass, n_cores: int):
        bass2jax.install_neuronx_cc_hook()
        assert nc.dbg_addr is None or not nc.dbg_callbacks
        partition_name = (
            nc.partition_id_tensor.name if nc.partition_id_tensor else None
        )
        in_names, out_names, out_avals = [], [], []
        for alloc in nc.m.functions[0].allocations:
            if not isinstance(alloc, mybir.MemoryLocationSet):
                continue
            name = alloc.memorylocations[0].name
            if alloc.kind == "ExternalInput":
                if name != partition_name and name != getattr(nc.dbg_addr, "name", None):
                    in_names.append(name)
            elif alloc.kind == "ExternalOutput":
                shape = tuple(alloc.tensor_shape)
                dtype = mybir.dt.np(alloc.dtype)
                out_names.append(name)
                out_avals.append(jax.core.ShapedArray(shape, dtype))
        self.in_names = list(in_names)
        self.out_names = out_names
        self.out_avals = out_avals
        self.n_cores = n_cores
        n_params = len(in_names)
        n_outs = len(out_avals)

        all_in_names = list(in_names) + list(out_names)
        if nc.dbg_addr is not None:
            all_in_names.append(nc.dbg_addr.name)
        if partition_name is not None:
            all_in_names.append(partition_name)
        self._has_dbg = nc.dbg_addr is not None

        def jnp_zeros_dbg():
            import jax.numpy as jnp

            return jnp.zeros((1, 2), np.uint32)

        def _call_once(ins, outs):
            operands = list(ins) + list(outs)
            if self._has_dbg:
                operands.append(jnp_zeros_dbg())
            if partition_name is not None:
                operands.append(bass2jax.partition_id_tensor())
            return tuple(
                bass2jax._bass_exec_p.bind(
                    *operands,
                    out_avals=tuple(out_avals),
                    in_names=tuple(all_in_names),
                    out_names=tuple(out_names),
                    lowering_input_output_aliases=(),
                    sim_require_finite=True,
                    sim_require_nnan=True,
                    nc=nc,
                )
            )

        self._call_once = _call_once

        def _body(*args):
            return _call_once(args[:n_params], args[n_params:])

        devices = jax.devices()[:n_cores]
        assert len(devices) == n_cores
        self.mesh = Mesh(np.asarray(devices), ("core",))
        in_specs = (PartitionSpec("core"),) * (n_params + n_outs)
        out_specs = (PartitionSpec("core"),) * n_outs
        donate = tuple(range(n_params, n_params + n_outs))
        self._fn = jax.jit(
            shard_map(
                _body,
                mesh=self.mesh,
                in_specs=in_specs,
                out_specs=out_specs,
                check_rep=False,
            ),
            donate_argnums=donate,
            keep_unused=True,
        )
        self.sharding = NamedSharding(self.mesh, PartitionSpec("core"))
        self._n_params = n_params
        self._n_outs = n_outs
        self._in_specs = in_specs
        self._chain_cache = {}

    def chain_fn(self, n: int):
        """Jitted fn executing the NEFF n times, serialized via the out bufs."""
        if n not in self._chain_cache:
            def _chain_body(*args):
                ins = args[: self._n_params]
                outs = tuple(args[self._n_params :])
                for _ in range(n):
                    outs = self._call_once(ins, outs)
                return outs

            donate = tuple(range(self._n_params, self._n_params + self._n_outs))
            self._chain_cache[n] = jax.jit(
                shard_map(
                    _chain_body,
                    mesh=self.mesh,
                    in_specs=self._in_specs,
                    out_specs=(PartitionSpec("core"),) * self._n_outs,
                    check_rep=False,
                ),
                donate_argnums=donate,
                keep_unused=True,
            )
        return self._chain_cache[n]

    def time_chain(self, concat_in_dev, n: int, reps: int = 10):
        import time

        fn = self.chain_fn(n)
        for _ in range(2):
            jax.block_until_ready(fn(*concat_in_dev, *self.fresh_zeros()))
        times = []
        for _ in range(reps):
            zeros = self.fresh_zeros()
            jax.block_until_ready(zeros)
            t0 = time.perf_counter()
            jax.block_until_ready(fn(*concat_in_dev, *zeros))
            times.append(time.perf_counter() - t0)
        return min(times)

    def concat_inputs(self, in_maps):
        return [
            np.concatenate([np.asarray(m[name]) for m in in_maps], axis=0)
            for name in self.in_names
        ]

    def fresh_zeros(self):
        return [
            jax.device_put(
                np.zeros((self.n_cores * a.shape[0], *a.shape[1:]), a.dtype),
                self.sharding,
            )
            for a in self.out_avals
        ]

    def __call__(self, concat_in):
        out_arrs = self._fn(*concat_in, *self.fresh_zeros())
        return [np.asarray(o) for o in out_arrs]


_EXEC_CACHE = {}


def _get_exec(shard: int, inner_reps: int = 1, variant: str = "v10-k0") -> _Exec:
    key = (shard, inner_reps, variant)
    if key not in _EXEC_CACHE:
        _EXEC_CACHE[key] = _Exec(_build(shard, inner_reps, variant=variant), N_CORES)
    return _EXEC_CACHE[key]


def _benchmark(mask_combined, edge_map, mask_index=1, inner_reps=257, reps=40):
    """Measure steady-state per-pass device time by comparing a 1-rep NEFF
    against an inner_reps-rep NEFF. Calls are interleaved pairwise and the
    median difference is used, cancelling the multi-ms drift of the ~80 ms
    axon dispatch overhead."""
    import time

    full, shard = _prep_inputs(mask_combined, edge_map, mask_index)
    ex1 = _get_exec(SHARD_PAD, 1)
    exR = _get_exec(SHARD_PAD, inner_reps)
    dev_in = [jax.device_put(full[name], ex1.sharding) for name in ex1.in_names]
    fn1 = ex1.chain_fn(1)
    fnR = exR.chain_fn(1)

    def one(ex, fn):
        z = ex.fresh_zeros()
        jax.block_until_ready(z)
        t0 = time.perf_counter()
        jax.block_until_ready(fn(*dev_in, *z))
        return time.perf_counter() - t0

    for _ in range(3):
        one(ex1, fn1)
        one(exR, fnR)
    diffs = []
    t1s = []
    for _ in range(reps):
        a = one(ex1, fn1)
        b = one(exR, fnR)
        t1s.append(a)
        diffs.append(b - a)
    diffs = np.array(diffs)
    loop_ns = float(np.median(diffs)) / (inner_reps - 1) * 1e9
    return {
        "dispatch_1rep_ns": float(np.min(t1s)) * 1e9,
        "diff_med_ns": float(np.median(diffs)) * 1e9,
        "diff_p25_ns": float(np.percentile(diffs, 25)) * 1e9,
        "diff_p75_ns": float(np.percentile(diffs, 75)) * 1e9,
        "loop_ns": loop_ns,
    }


def _prep_inputs(mask_combined, edge_map, mask_index):
    import ml_dtypes

    bf16 = ml_dtypes.bfloat16
    idx = int(np.asarray(mask_index))
    B = mask_combined.shape[0]
    assert B % N_CORES == 0, B
    shard = B // N_CORES
    assert shard <= SHARD_PAD
    mc = np.asarray(mask_combined[..., idx], dtype=np.float32).astype(bf16)
    mc[:, :, -1] = 0  # reference zeroes last col/row of the selected mask
    mc[:, -1, :] = 0
    mc = mc.reshape(B, AREA)
    em = (
        np.asarray(edge_map, dtype=np.float32)[..., 0]
        .reshape(B, AREA)[:, :NV]
        .astype(bf16)
    )
    # pad each core's shard to SHARD_PAD rows of zeros (zero areas -> zero loss)
    def pad(x):
        x = x.reshape(N_CORES, shard, x.shape[-1])
        out = np.zeros((N_CORES, SHARD_PAD, x.shape[-1]), x.dtype)
        out[:, :shard] = x
        return out.reshape(N_CORES * SHARD_PAD, x.shape[-1])

    return {"mc": pad(mc), "em": pad(em)}, shard


def _run(resized_image=None, mask_combined=None, edge_map=None, mask_index=1, **_):
    full, shard = _prep_inputs(mask_combined, edge_map, mask_index)
    ex = _get_exec(SHARD_PAD)
    concat_in = [full[name] for name in ex.in_names]
    outs = ex(concat_in)
    out = outs[ex.out_names.index("out")].reshape(N_CORES, SHARD_PAD)[:, :shard]
    return out.reshape(-1).astype(np.float32, copy=False), ex


def kernel(**inputs) -> np.ndarray:
    out, _ = _run(**inputs)
    return out


def _time_reps(resized_image=None, mask_combined=None, edge_map=None, mask_index=1, reps=30, **_):
    import time

    full, shard = _prep_inputs(mask_combined, edge_map, mask_index)
    ex = _get_exec(shard)
    concat_in = [
        jax.device_put(full[name], ex.sharding) for name in ex.in_names
    ]
    for _i in range(3):
        jax.block_until_ready(ex._fn(*concat_in, *ex.fresh_zeros()))
    times = []
    for _i in range(reps):
        zeros = ex.fresh_zeros()
        jax.block_until_ready(zeros)
        t0 = time.perf_counter()
        jax.block_until_ready(ex._fn(*concat_in, *zeros))
        times.append(time.perf_counter() - t0)
    return times


def _build_null() -> bass.Bass:
    nc = bacc.Bacc("TRN2", target_bir_lowering=False, debug=False)
    x_d = nc.declare_dram_parameter("x", [128, 8], F32, isOutput=False)
    y_d = nc.declare_dram_parameter("y", [128, 8], F32, isOutput=True)
    with tile.TileContext(nc) as tc:
        with tc.tile_pool(name="p", bufs=1) as pool:
            t = pool.tile([128, 8], F32)
            nc.sync.dma_start(out=t[:], in_=x_d.ap()[:])
            nc.sync.dma_start(out=y_d.ap()[:], in_=t[:])
    nc.compile()
    return nc


def _time_null(reps=30):
    import time

    if "null" not in _EXEC_CACHE:
        _EXEC_CACHE["null"] = _Exec(_build_null(), N_CORES)
    ex = _EXEC_CACHE["null"]
    x = np.zeros((N_CORES * 128, 8), np.float32)
    concat_in = [jax.device_put(x, ex.sharding)]
    for _i in range(3):
        jax.block_until_ready(ex._fn(*concat_in, *ex.fresh_zeros()))
    times = []
    for _i in range(reps):
        zeros = ex.fresh_zeros()
        jax.block_until_ready(zeros)
        t0 = time.perf_counter()
        jax.block_until_ready(ex._fn(*concat_in, *zeros))
        times.append(time.perf_counter() - t0)
    return times


# revision 39
# speedup vs baseline: 1.0345x; 1.0345x over previous
"""Trainium2 Bass kernel for nn_Apply_on_single_area.

Computes, per supervoxel area b:
    loss[b] = sum_{i,j} eroded(mc)[i,j] * em[i,j]
where mc = mask_combined[..., mask_index] with last row/col zeroed and
eroded = E(a1)*E(a2), E(a) = 2a - a^2, a1/a2 = products with the next
element along each spatial axis (zero-padded).

Key simplifications / design (HW-measured on TRN2):
- differentiable_or_simple(a,b) = a*b + (1-a)*a + (1-b)*a = 2a - a^2:
  the b-terms cancel, so only forward-neighbor products a1, a2 matter.
- Flattening (i,j) -> k = 32i+j turns the spatial shifts into flat
  shifts +32/+1; zeroing row/col 31 of mc makes that exact.
- Pure data parallel: B=10000 split 1250/core over 8 cores, padded to
  1280 = 128 partitions x 10 areas, partition-major so every DMA is
  contiguous per partition.
- bf16 compute (inputs converted host-side; rel err ~1e-3 vs the 2e-2
  gate), f32 accumulation for the per-area sums.
- Engine split: DVE does the same-tensor shifted products t1,t2 (2x
  mode), the fused e12 = 1-u12 tensor_scalar, p = e1*e2 and w = p*em;
  ACT does one fused Square for u12 = (1-t12)^2 and the per-area
  Copy+accum reductions. GPSIMD is deliberately unused - any gpsimd
  involvement measured ~10us slower. Emission is lag-pipelined
  (stage_a/b/c offset by supertile) so neither engine stalls on
  same-supertile cross-engine dependencies.
"""

import numpy as np

import jax
from jax.experimental.shard_map import shard_map
from jax.sharding import Mesh, NamedSharding, PartitionSpec

import concourse.bass as bass
import concourse.bacc as bacc
import concourse.mybir as mybir
import concourse.tile as tile
from concourse import bass2jax

N_CORES = 8
B_TOTAL = 10000
SHARD = B_TOTAL // N_CORES  # 1250
C_PER_P = 10  # areas per partition (after padding shard to 1280)
SHARD_PAD = 128 * C_PER_P
AREA = 1024  # 32*32
W = 32
NV = AREA - W  # 992 valid flat positions (rows 0..30)

F32 = mybir.dt.float32
BF16 = mybir.dt.bfloat16

_NC_CACHE = {}


def _supertiles(shard: int, A: int):
    """Split `shard` areas into supertiles (base, P, a) with a area-slots of
    P partitions each. Area index = base + 128*j + p for slot j, partition p."""
    out = []
    base = 0
    while shard - base >= 128 * A:
        out.append((base, 128, A))
        base += 128 * A
    while shard - base >= 128:
        out.append((base, 128, 1))
        base += 128
    if shard > base:
        out.append((base, shard - base, 1))
        base = shard
    return out


def _build(shard: int, inner_reps: int = 1, A: int = 2, variant: str = "v10-k0") -> bass.Bass:
    """Per-core SPMD graph: mc [1280,1024] bf16 (edges pre-zeroed, rows
    1250..1279 zero-padded), em [1280,992] bf16 -> out [1280] f32.

    Partition-major layout: area = p*C_PER_P + t, so every DMA is
    contiguous per partition (loads 2-4 KB lines, store one 40 B line).

    Math: loss = sum_k e(t1)*e(t2)*em with e(t) = t*(2-t) = 1-(1-t)^2,
    t1[k]=m[k]*m[k+32], t2[k]=m[k]*m[k+1] over k in [0,992).

    Two-engine split (HW-measured): DVE t1/t2 (same-tensor shifted tt),
    e=1-u (ts), p=e1*e2, w=p*em (tt); ACT squares u=(1-t)^2 and the
    final Copy+accum reduction per area. Lag-pipelined emission."""
    assert shard == SHARD_PAD, shard
    C = C_PER_P
    nc = bacc.Bacc("TRN2", target_bir_lowering=False, debug=False)

    mc_d = nc.declare_dram_parameter("mc", [shard, AREA], BF16, isOutput=False)
    em_d = nc.declare_dram_parameter("em", [shard, NV], BF16, isOutput=False)
    out_d = nc.declare_dram_parameter("out", [shard], F32, isOutput=True)

    n_super = C // A
    AL = mybir.AluOpType
    AF = mybir.ActivationFunctionType
    mc_v = mc_d.ap().rearrange("(p c) k -> p c k", c=C)
    em_v = em_d.ap().rearrange("(p c) k -> p c k", c=C)

    reuse = "reuse" in variant or "bufs6" in variant
    mid_bufs = 6 if "bufs6" in variant else (3 if "mix" in variant else 4)
    lag_c = 3 if "lag3" in variant else 2
    ld_bufs = 6 if "ldb6" in variant else 4
    with tile.TileContext(nc) as tc:
        with (
            tc.tile_pool(name="ld", bufs=ld_bufs) as ld,
            tc.tile_pool(name="mid", bufs=mid_bufs) as mid,
            tc.tile_pool(name="res", bufs=4) as resp,
            tc.tile_pool(name="stat", bufs=1) as statp,
        ):
          if "mix" in variant and inner_reps:
            stat = {}
            for nm, shp in [("sm", [128, A, AREA]), ("se", [128, A, NV]),
                            ("st", [128, 2, A, NV]), ("su", [128, 2, A, NV]),
                            ("sе12", [128, 2, A, NV]), ("sp", [128, A, NV]),
                            ("sw", [128, A, NV])]:
                t = statp.tile(shp, BF16, tag="stat_" + nm)
                nc.vector.memset(t[:], 0.25)
                stat[nm] = t
          for _rr in range(inner_reps):
            res_t = resp.tile([128, C], F32, tag="res")
            stage_state = {}
            if "mix" in variant:
                for s in range(n_super):
                    c0 = s * A
                    m = ld.tile([128, A, AREA], BF16, tag="m")
                    nc.sync.dma_start(out=m[:], in_=mc_v[:, c0 : c0 + A, :])
                    e = ld.tile([128, A, NV], BF16, tag="e")
                    nc.sync.dma_start(out=e[:], in_=em_v[:, c0 : c0 + A, :])
                    t12 = mid.tile([128, 2, A, NV], BF16, tag="t12")
                    sm = stat["sm"]
                    nc.vector.tensor_tensor(t12[:, 0], sm[:, :, 0:NV], sm[:, :, W:AREA], AL.mult)
                    nc.vector.tensor_tensor(t12[:, 1], sm[:, :, 0:NV], sm[:, :, 1 : 1 + NV], AL.mult)
                    u12 = mid.tile([128, 2, A, NV], BF16, tag="u12")
                    nc.scalar.activation(u12[:], stat["st"][:], AF.Square, bias=1.0, scale=-1.0)
                    e12 = mid.tile([128, 2, A, NV], BF16, tag="e12")
                    nc.vector.tensor_scalar(e12[:], stat["su"][:], -1.0, 1.0, op0=AL.mult, op1=AL.add)
                    p_t = mid.tile([128, A, NV], BF16, tag="p")
                    se12 = stat["sе12"]
                    nc.vector.tensor_tensor(p_t[:], se12[:, 0], se12[:, 1], AL.mult)
                    w = mid.tile([128, A, NV], BF16, tag="w")
                    nc.vector.tensor_tensor(w[:], stat["sp"][:], stat["se"][:], AL.mult)
                    for j in range(A):
                        dum = mid.tile([128, NV], BF16, tag="dum")
                        nc.scalar.activation(
                            dum[:], stat["sw"][:, j], AF.Copy,
                            accum_out=res_t[:, c0 + j : c0 + j + 1],
                        )
                nc.sync.dma_start(
                    out=out_d.ap().rearrange("(p c) -> p c", c=C), in_=res_t[:]
                )
                continue

            def stage_a(s):
                c0 = s * A
                m = ld.tile([128, A, AREA], BF16, tag="m")
                e = ld.tile([128, A, NV], BF16, tag="e")
                if variant == "tinydma":
                    nc.sync.dma_start(out=m[:, :, 0:16], in_=mc_v[:, c0 : c0 + A, 0:16])
                    nc.sync.dma_start(out=e[:, :, 0:16], in_=em_v[:, c0 : c0 + A, 0:16])
                else:
                    nc.sync.dma_start(out=m[:], in_=mc_v[:, c0 : c0 + A, :])
                    nc.sync.dma_start(out=e[:], in_=em_v[:, c0 : c0 + A, :])
                if variant == "dmaonly":
                    stage_state[s] = (e, None, None)
                    return
                if "v10" in variant or "v13" in variant:
                    t12 = mid.tile([128, 2, A, NV], BF16, tag="t12")
                    nc.vector.tensor_tensor(
                        t12[:, 0], m[:, :, 0:NV], m[:, :, W:AREA], AL.mult
                    )
                    nc.vector.tensor_tensor(
                        t12[:, 1], m[:, :, 0:NV], m[:, :, 1 : 1 + NV], AL.mult
                    )
                    stage_state[s] = (e, t12, None)
                    return
                t1 = mid.tile([128, A, NV], BF16, tag="t1")
                nc.vector.tensor_tensor(t1[:], m[:, :, 0:NV], m[:, :, W:AREA], AL.mult)
                t2 = mid.tile([128, A, NV], BF16, tag="t2")
                nc.vector.tensor_tensor(t2[:], m[:, :, 0:NV], m[:, :, 1 : 1 + NV], AL.mult)
                stage_state[s] = (e, t1, t2)

            def stage_b(s):
                if variant == "dmaonly":
                    return
                e, t1, t2 = stage_state[s]
                if "v10" in variant or "v13" in variant:
                    t12 = t1
                    u12 = mid.tile([128, 2, A, NV], BF16, tag="u12")
                    nc.scalar.activation(u12[:], t12[:], AF.Square, bias=1.0, scale=-1.0)
                    e12 = mid.tile([128, 2, A, NV], BF16, tag="e12")
                    nc.vector.tensor_scalar(
                        e12[:], u12[:], -1.0, 1.0, op0=AL.mult, op1=AL.add
                    )
                    stage_state[s] = (e, e12, None)
                    return
                u1 = mid.tile([128, A, NV], BF16, tag="u1")
                nc.scalar.activation(u1[:], t1[:], AF.Square, bias=1.0, scale=-1.0)
                u2 = mid.tile([128, A, NV], BF16, tag="u2")
                nc.scalar.activation(u2[:], t2[:], AF.Square, bias=1.0, scale=-1.0)
                if "v9" in variant:
                    # e1,e2 share one tile so p = e1*e2 hits the DVE
                    # single-source fastpath
                    e12 = mid.tile([128, 2, A, NV], BF16, tag="e12")
                    nc.vector.tensor_scalar(
                        e12[:, 0], u1[:], -1.0, 1.0, op0=AL.mult, op1=AL.add
                    )
                    nc.vector.tensor_scalar(
                        e12[:, 1], u2[:], -1.0, 1.0, op0=AL.mult, op1=AL.add
                    )
                    stage_state[s] = (e, e12, None)
                    return
                e1 = mid.tile([128, A, NV], BF16, tag="t1" if reuse else "e1")
                nc.vector.tensor_scalar(e1[:], u1[:], -1.0, 1.0, op0=AL.mult, op1=AL.add)
                e2 = mid.tile([128, A, NV], BF16, tag="t2" if reuse else "e2")
                nc.vector.tensor_scalar(e2[:], u2[:], -1.0, 1.0, op0=AL.mult, op1=AL.add)
                stage_state[s] = (e, e1, e2)

            def stage_c(s):
                c0 = s * A
                e, e1, e2 = stage_state.pop(s)
                if variant == "dmaonly":
                    return
                if "v13" in variant:
                    # total-work-lean: per-slot stt fuses w=p*em with the
                    # f32 accumulate; no w tile, no ACT reductions
                    e12 = e1
                    p_t = mid.tile([128, A, NV], BF16, tag="p")
                    nc.vector.tensor_tensor(p_t[:], e12[:, 0], e12[:, 1], AL.mult)
                    for j in range(A):
                        qd = mid.tile([128, NV], BF16, tag="dum")
                        nc.vector.scalar_tensor_tensor(
                            qd[:], p_t[:, j], 1.0, e[:, j],
                            op0=AL.mult, op1=AL.mult,
                            accum_out=res_t[:, c0 + j : c0 + j + 1],
                        )
                    return
                if "v9" in variant or "v10" in variant:
                    e12 = e1
                    p_t = mid.tile([128, A, NV], BF16, tag="p")
                    nc.vector.tensor_tensor(p_t[:], e12[:, 0], e12[:, 1], AL.mult)
                    w = mid.tile([128, A, NV], BF16, tag="w")
                    nc.vector.tensor_tensor(w[:], p_t[:], e[:], AL.mult)
                    if "fold1" in variant:
                        # halve reduction length with one same-tile tt-add
                        wh = mid.tile([128, A, NV // 2], BF16, tag="wh")
                        nc.vector.tensor_tensor(
                            wh[:], w[:, :, 0 : NV // 2], w[:, :, NV // 2 : NV], AL.add
                        )
                        for j in range(A):
                            dum = mid.tile([128, NV // 2], BF16, tag="dumh")
                            nc.scalar.activation(
                                dum[:], wh[:, j], AF.Copy,
                                accum_out=res_t[:, c0 + j : c0 + j + 1],
                            )
                        return
                    for j in range(A):
                        if "k0" not in variant and s in (1, 3) and j == 0:
                            qd = mid.tile([128, NV], BF16, tag="dum")
                            nc.vector.tensor_scalar(
                                qd[:], w[:, j], 1.0, None, op0=AL.mult, op1=AL.add,
                                accum_out=res_t[:, c0 + j : c0 + j + 1],
                            )
                        else:
                            dum = mid.tile([128, NV], BF16, tag="dum")
                            nc.scalar.activation(
                                dum[:], w[:, j], AF.Copy,
                                accum_out=res_t[:, c0 + j : c0 + j + 1],
                            )
                    return
                p_t = mid.tile([128, A, NV], BF16, tag="u1" if reuse else "p")
                nc.vector.tensor_tensor(p_t[:], e1[:], e2[:], AL.mult)
                if "v8" in variant:
                    # w = p*em on DVE, then DMA-CCE tree-fold (add) 992->124
                    # so the per-area reductions touch only 124 elements
                    w = mid.tile([128, A, NV], BF16, tag="u2" if reuse else "w")
                    nc.vector.tensor_tensor(w[:], p_t[:], e[:], AL.mult)
                    nc.gpsimd.dma_start(
                        out=w[:, :, 0:496], in_=w[:, :, 496:992], accum_op=AL.add
                    )
                    nc.gpsimd.dma_start(
                        out=w[:, :, 0:248], in_=w[:, :, 248:496], accum_op=AL.add
                    )
                    nc.gpsimd.dma_start(
                        out=w[:, :, 0:124], in_=w[:, :, 124:248], accum_op=AL.add
                    )
                    for j in range(A):
                        dum = mid.tile([128, 124], BF16, tag="dum8")
                        nc.scalar.activation(
                            dum[:], w[:, j, 0:124], AF.Copy,
                            accum_out=res_t[:, c0 + j : c0 + j + 1],
                        )
                    return
                if "v7" in variant:
                    # w = p*em computed by the DMA compute engine, in-place
                    # into the em tile; reductions split DVE/ACT per slot
                    nc.gpsimd.dma_start(out=e[:], in_=p_t[:], accum_op=AL.mult)
                    for j in range(A):
                        if j % 2 == 0:
                            qd = mid.tile([128, NV], BF16, tag="dum")
                            nc.vector.tensor_scalar(
                                qd[:], e[:, j], 1.0, None, op0=AL.mult,
                                op1=AL.add,
                                accum_out=res_t[:, c0 + j : c0 + j + 1],
                            )
                        else:
                            dum = mid.tile([128, NV], BF16, tag="dum")
                            nc.scalar.activation(
                                dum[:], e[:, j], AF.Copy,
                                accum_out=res_t[:, c0 + j : c0 + j + 1],
                            )
                    return
                w = mid.tile([128, A, NV], BF16, tag="u2" if reuse else "w")
                if "gw" in variant:
                    nc.gpsimd.tensor_tensor(w[:], p_t[:], e[:], AL.mult)
                else:
                    nc.vector.tensor_tensor(w[:], p_t[:], e[:], AL.mult)
                for j in range(A):
                    dum = mid.tile([128, NV], BF16, tag="dum")
                    nc.scalar.activation(
                        dum[:], w[:, j], AF.Copy,
                        accum_out=res_t[:, c0 + j : c0 + j + 1],
                    )

            for s in range(n_super + lag_c):
                if s < n_super:
                    stage_a(s)
                if 1 <= s < n_super + 1:
                    stage_b(s - 1)
                if s >= lag_c:
                    stage_c(s - lag_c)

            if variant == "dmaonly":
                nc.gpsimd.memset(res_t[:], 0.0)
            if variant != "lastout" or _rr == inner_reps - 1:
                nc.sync.dma_start(
                    out=out_d.ap().rearrange("(p c) -> p c", c=C), in_=res_t[:]
                )

    nc.compile()
    return nc

class _Exec:
    """One-time-jitted SPMD executor for a prebuilt Bass graph.

    Vendored from bass2jax.run_bass_via_pjrt so repeated calls reuse the
    compiled executable (run_bass_via_pjrt re-jits per invocation)."""

    def __init__(self, nc: bass.Bass, n_cores: int):
        bass2jax.install_neuronx_cc_hook()
        assert nc.dbg_addr is None or not nc.dbg_callbacks
        partition_name = (
            nc.partition_id_tensor.name if nc.partition_id_tensor else None
        )
        in_names, out_names, out_avals = [], [], []
        for alloc in nc.m.functions[0].allocations:
            if not isinstance(alloc, mybir.MemoryLocationSet):
                continue
            name = alloc.memorylocations[0].name
            if alloc.kind == "ExternalInput":
                if name != partition_name and name != getattr(nc.dbg_addr, "name", None):
                    in_names.append(name)
            elif alloc.kind == "ExternalOutput":
                shape = tuple(alloc.tensor_shape)
                dtype = mybir.dt.np(alloc.dtype)
                out_names.append(name)
                out_avals.append(jax.core.ShapedArray(shape, dtype))
        self.in_names = list(in_names)
        self.out_names = out_names
        self.out_avals = out_avals
        self.n_cores = n_cores
        n_params = len(in_names)
        n_outs = len(out_avals)

        all_in_names = list(in_names) + list(out_names)
        if nc.dbg_addr is not None:
            all_in_names.append(nc.dbg_addr.name)
        if partition_name is not None:
            all_in_names.append(partition_name)
        self._has_dbg = nc.dbg_addr is not None

        def jnp_zeros_dbg():
            import jax.numpy as jnp

            return jnp.zeros((1, 2), np.uint32)

        def _call_once(ins, outs):
            operands = list(ins) + list(outs)
            if self._has_dbg:
                operands.append(jnp_zeros_dbg())
            if partition_name is not None:
                operands.append(bass2jax.partition_id_tensor())
            return tuple(
                bass2jax._bass_exec_p.bind(
                    *operands,
                    out_avals=tuple(out_avals),
                    in_names=tuple(all_in_names),
                    out_names=tuple(out_names),
                    lowering_input_output_aliases=(),
                    sim_require_finite=True,
                    sim_require_nnan=True,
                    nc=nc,
                )
            )

        self._call_once = _call_once

        def _body(*args):
            return _call_once(args[:n_params], args[n_params:])

        devices = jax.devices()[:n_cores]
        assert len(devices) == n_cores
        self.mesh = Mesh(np.asarray(devices), ("core",))
        in_specs = (PartitionSpec("core"),) * (n_params + n_outs)
        out_specs = (PartitionSpec("core"),) * n_outs
        donate = tuple(range(n_params, n_params + n_outs))
        self._fn = jax.jit(
            shard_map(
                _body,
                mesh=self.mesh,
                in_specs=in_specs,
                out_specs=out_specs,
                check_rep=False,
            ),
            donate_argnums=donate,
            keep_unused=True,
        )
        self.sharding = NamedSharding(self.mesh, PartitionSpec("core"))
        self._n_params = n_params
        self._n_outs = n_outs
        self._in_specs = in_specs
        self._chain_cache = {}

    def chain_fn(self, n: int):
        """Jitted fn executing the NEFF n times, serialized via the out bufs."""
        if n not in self._chain_cache:
            def _chain_body(*args):
                ins = args[: self._n_params]
                outs = tuple(args[self._n_params :])
                for _ in range(n):
                    outs = self._call_once(ins, outs)
                return outs

            donate = tuple(range(self._n_params, self._n_params + self._n_outs))
            self._chain_cache[n] = jax.jit(
                shard_map(
                    _chain_body,
                    mesh=self.mesh,
                    in_specs=self._in_specs,
                    out_specs=(PartitionSpec("core"),) * self._n_outs,
                    check_rep=False,
                ),
                donate_argnums=donate,
                keep_unused=True,
            )
        return self._chain_cache[n]

    def time_chain(self, concat_in_dev, n: int, reps: int = 10):
        import time

        fn = self.chain_fn(n)
        for _ in range(2):
            jax.block_until_ready(fn(*concat_in_dev, *self.fresh_zeros()))
        times = []
        for _ in range(reps):
            zeros = self.fresh_zeros()
            jax.block_until_ready(zeros)
            t0 = time.perf_counter()
            jax.block_until_ready(fn(*concat_in_dev, *zeros))
            times.append(time.perf_counter() - t0)
        return min(times)

    def concat_inputs(self, in_maps):
        return [
            np.concatenate([np.asarray(m[name]) for m in in_maps], axis=0)
            for name in self.in_names
        ]

    def fresh_zeros(self):
        return [
            jax.device_put(
                np.zeros((self.n_cores * a.shape[0], *a.shape[1:]), a.dtype),
                self.sharding,
            )
            for a in self.out_avals
        ]

    def __call__(self, concat_in):
        out_arrs = self._fn(*concat_in, *self.fresh_zeros())
        return [np.asarray(o) for o in out_arrs]


_EXEC_CACHE = {}


def _get_exec(shard: int, inner_reps: int = 1, variant: str = "v10-k0") -> _Exec:
    key = (shard, inner_reps, variant)
    if key not in _EXEC_CACHE:
        _EXEC_CACHE[key] = _Exec(_build(shard, inner_reps, variant=variant), N_CORES)
    return _EXEC_CACHE[key]


def _benchmark(mask_combined, edge_map, mask_index=1, inner_reps=257, reps=40):
    """Measure steady-state per-pass device time by comparing a 1-rep NEFF
    against an inner_reps-rep NEFF. Calls are interleaved pairwise and the
    median difference is used, cancelling the multi-ms drift of the ~80 ms
    axon dispatch overhead."""
    import time

    full, shard = _prep_inputs(mask_combined, edge_map, mask_index)
    ex1 = _get_exec(SHARD_PAD, 1)
    exR = _get_exec(SHARD_PAD, inner_reps)
    dev_in = [jax.device_put(full[name], ex1.sharding) for name in ex1.in_names]
    fn1 = ex1.chain_fn(1)
    fnR = exR.chain_fn(1)

    def one(ex, fn):
        z = ex.fresh_zeros()
        jax.block_until_ready(z)
        t0 = time.perf_counter()
        jax.block_until_ready(fn(*dev_in, *z))
        return time.perf_counter() - t0

    for _ in range(3):
        one(ex1, fn1)
        one(exR, fnR)
    diffs = []
    t1s = []
    for _ in range(reps):
        a = one(ex1, fn1)
        b = one(exR, fnR)
        t1s.append(a)
        diffs.append(b - a)
    diffs = np.array(diffs)
    loop_ns = float(np.median(diffs)) / (inner_reps - 1) * 1e9
    return {
        "dispatch_1rep_ns": float(np.min(t1s)) * 1e9,
        "diff_med_ns": float(np.median(diffs)) * 1e9,
        "diff_p25_ns": float(np.percentile(diffs, 25)) * 1e9,
        "diff_p75_ns": float(np.percentile(diffs, 75)) * 1e9,
        "loop_ns": loop_ns,
    }


def _prep_inputs(mask_combined, edge_map, mask_index):
    import ml_dtypes

    bf16 = ml_dtypes.bfloat16
    idx = int(np.asarray(mask_index))
    B = mask_combined.shape[0]
    assert B % N_CORES == 0, B
    shard = B // N_CORES
    assert shard <= SHARD_PAD
    mc = np.asarray(mask_combined[..., idx], dtype=np.float32).astype(bf16)
    mc[:, :, -1] = 0  # reference zeroes last col/row of the selected mask
    mc[:, -1, :] = 0
    mc = mc.reshape(B, AREA)
    em = (
        np.asarray(edge_map, dtype=np.float32)[..., 0]
        .reshape(B, AREA)[:, :NV]
        .astype(bf16)
    )
    # pad each core's shard to SHARD_PAD rows of zeros (zero areas -> zero loss)
    def pad(x):
        x = x.reshape(N_CORES, shard, x.shape[-1])
        out = np.zeros((N_CORES, SHARD_PAD, x.shape[-1]), x.dtype)
        out[:, :shard] = x
        return out.reshape(N_CORES * SHARD_PAD, x.shape[-1])

    return {"mc": pad(mc), "em": pad(em)}, shard


def _run(resized_image=None, mask_combined=None, edge_map=None, mask_index=1, **_):
    full, shard = _prep_inputs(mask_combined, edge_map, mask_index)
    ex = _get_exec(SHARD_PAD)
    concat_in = [full[name] for name in ex.in_names]
    outs = ex(concat_in)
    out = outs[ex.out_names.index("out")].reshape(N_CORES, SHARD_PAD)[:, :shard]
    return out.reshape(-1).astype(np.float32, copy=False), ex


def kernel(**inputs) -> np.ndarray:
    out, _ = _run(**inputs)
    return out


def _time_reps(resized_image=None, mask_combined=None, edge_map=None, mask_index=1, reps=30, **_):
    import time

    full, shard = _prep_inputs(mask_combined, edge_map, mask_index)
    ex = _get_exec(shard)
    concat_in = [
        jax.device_put(full[name], ex.sharding) for name in ex.in_names
    ]
    for _i in range(3):
        jax.block_until_ready(ex._fn(*concat_in, *ex.fresh_zeros()))
    times = []
    for _i in range(reps):
        zeros = ex.fresh_zeros()
        jax.block_until_ready(zeros)
        t0 = time.perf_counter()
        jax.block_until_ready(ex._fn(*concat_in, *zeros))
        times.append(time.perf_counter() - t0)
    return times


def _build_null() -> bass.Bass:
    nc = bacc.Bacc("TRN2", target_bir_lowering=False, debug=False)
    x_d = nc.declare_dram_parameter("x", [128, 8], F32, isOutput=False)
    y_d = nc.declare_dram_parameter("y", [128, 8], F32, isOutput=True)
    with tile.TileContext(nc) as tc:
        with tc.tile_pool(name="p", bufs=1) as pool:
            t = pool.tile([128, 8], F32)
            nc.sync.dma_start(out=t[:], in_=x_d.ap()[:])
            nc.sync.dma_start(out=y_d.ap()[:], in_=t[:])
    nc.compile()
    return nc


def _time_null(reps=30):
    import time

    if "null" not in _EXEC_CACHE:
        _EXEC_CACHE["null"] = _Exec(_build_null(), N_CORES)
    ex = _EXEC_CACHE["null"]
    x = np.zeros((N_CORES * 128, 8), np.float32)
    concat_in = [jax.device_put(x, ex.sharding)]
    for _i in range(3):
        jax.block_until_ready(ex._fn(*concat_in, *ex.fresh_zeros()))
    times = []
    for _i in range(reps):
        zeros = ex.fresh_zeros()
        jax.block_until_ready(zeros)
        t0 = time.perf_counter()
        jax.block_until_ready(ex._fn(*concat_in, *zeros))
        times.append(time.perf_counter() - t0)
    return times
